# revision 59
# baseline (speedup 1.0000x reference)
"""Trainium2 Bass kernel for nn_DeformConv2d_72765335929324.

The module is a dense 3x3 conv (stride 1, pad 1) [B,64,256,256] -> [B,18,256,256]
plus a per-pixel additive `offset` term and a channel bias.

Strategy (per core; batch is sharded 2 images/core across 8 cores):
- conv = 9 taps, each a [cin=64 -> cout=18] matmul over shifted input views.
- PE array packing via tile_position: 2 images on row-groups {0,64} x 4
  output row-pair chunks on col-groups {0,32,64,96} -> 8 concurrent matmul
  streams, 9 accumulating taps each, N=512 (2 output rows) per stream.
- The moving operand (input) is float8e3: the PE runs fp8 at bf16 speed, so
  this halves input HBM traffic for free; weights stay bf16 (mixed-dtype
  matmul is supported and HW-exact vs the quantized reference). x is
  pre-scaled by 2 (w by 1/2) to keep small values out of e3m4 subnormals
  while staying under the e3m4 max normal of 15.5 (clipped for safety).
- offset+bias are pre-added on the host, packed into the SBUF tile layout
  and shipped as bf16; output is returned as bf16 (~2x less off/out HBM
  traffic vs f32, rel-err ~4.6e-3 vs the 2e-2 budget).
- All bulk transfers are full-width [128, *] DMAs split into ~0.5-2MB
  halves so they spread across DMA queues (A/B-tested: one big DMA per
  slab and/or moving stores to the ACT HWDGE ring both measured SLOWER).
- W-edge zero-padding is realized by shrinking the matmul N-span per kw tap;
  H-edge padding by shrinking the row span of the first/last chunk taps.
"""

import contextlib
import os
import numpy as np

import concourse.bass as bass
import concourse.tile as tile
import concourse.mybir as mybir
from concourse.vector_clock import ScopedClock
from concourse.bass_utils import run_bass_kernel_spmd

B, CIN, H, W = 16, 64, 256, 256
COUT = 18
COUTP = 32  # cout padded to a 32-wide PE column group
NCORES = 8
BPC = B // NCORES  # images per core
R = 64  # output rows per input slab
NSLAB = H // R
QPS = R // 8  # quad chunks per slab (each quad = 8 rows)
FQ = 2 * W  # free size of one quad chunk (2 rows x 256 cols)
FS = QPS * FQ  # free size of one slab tile

# tap order: full-coverage center tap first so start=True initializes the
# whole PSUM bank region before partial-coverage taps accumulate.
TAPS = [(1, 1), (0, 0), (0, 1), (0, 2), (1, 0), (1, 2), (2, 0), (2, 1), (2, 2)]

# kw -> (src col offset, dst col offset, ncols): zero-pad at W edges is
# realized by shrinking the span instead of padding SBUF.
KW_SPAN = {0: (0, 1, W - 1), 1: (0, 0, W), 2: (1, 0, W - 1)}

DT_NAME = os.environ.get("CONV_DT", "bfloat16")
# input (moving operand) dtype: float8e3 halves input HBM traffic; the
# PE runs fp8 at bf16 speed so only DMA changes. x is pre-scaled by XS
# (and w by 1/XS, exact in bf16) to push small values out of the e3m4
# subnormal range; e3m4 max normal is 31 so |4x| <= ~24 is safe.
X_DT = os.environ.get("CONV_XDT", "float8e3")
XS = 2.0  # e3m4 max normal is 15.5: |2x| <= ~11 for this data, clipped anyway
# timing experiments only: restrict the tap count (wrong results!)
N_TAPS = int(os.environ.get("CONV_TAPS", "9"))


class _TileContext(tile.TileContext):
    """TileContext whose tail drain spreads its semaphore waits over NOPs.

    The stock _drain_and_barrier puts one wait per logical proc on a single
    Drain instruction; the walrus build here rejects instructions carrying
    more than 1-2 sync waits.
    """

    def _drain_and_barrier(self, tick_clock, wait_clock):
        nc = self.nc
        carriers = [nc.sync.nop(nofuse=True) for _ in range(64)]
        drain_inst = nc.sync.drain()
        wait_clock.add_sem_waits(
            drain_inst.ins, ScopedClock({None: tick_clock.global_clock})
        )
        si = drain_inst.ins.sync_info
        waits = list(si.on_wait or []) if si is not None else []
        if len(waits) > 1:
            si.on_wait = waits[:1]
            extra = waits[1:]
            assert len(extra) <= len(carriers)
            for wt, nop in zip(extra, carriers):
                nsi = nop.ins.sync_info
                if nsi is None:
                    nop.ins.sync_info = mybir.SyncInfo(on_wait=[wt], on_update=[])
                else:
                    nsi.on_wait = [wt]
        nc.all_engine_barrier()
        assert self.sems is not None
        popped = nc._tile_sem_poison_stack.pop()
        assert popped is self._sem_poison
        nc.clear_and_free_semaphores(list(self.sems.allocated().values()))
        nc.all_engine_barrier()


def _split_excess_waits(nc):
    """Spill per-instruction semaphore waits onto same-engine NOP carriers.

    Tile's wait assigner attaches up to ~6 waits to one instruction; the
    walrus build here rejects >1 sync wait on engine instructions (>2 on
    EventSemaphore). A NOP that runs just before the instruction on the same
    engine is semantically equivalent (program order on one engine is
    serial). For DMAs, the wait kept in-descriptor is evaluated by the DGE
    without stalling the issuing engine, so keep the freshest (engine-sem)
    wait there and spill the long-satisfied WAR waits on old DMA completions.
    """
    for bb in nc.m.functions[0].blocks:
        new = []
        for inst in bb.instructions:
            si = inst.sync_info
            waits = list(si.on_wait) if si and si.on_wait else []
            cap = 2 if isinstance(inst, mybir.InstEventSemaphore) else 1
            if len(waits) > cap:
                if isinstance(inst, mybir.InstDMACopy):
                    waits.sort(key=lambda w: ((w.ant_name or "").startswith("DMA"),))
                si.on_wait = waits[:cap]
                for w in waits[cap:]:
                    n = mybir.InstNoOp(
                        name=nc.get_next_instruction_name(), ins=[], outs=[]
                    )
                    n.engine = inst.engine
                    n.sync_info = mybir.SyncInfo(on_wait=[w], on_update=[])
                    new.append(n)
            new.append(inst)
        bb.instructions = new


def build_nc(dt_name=DT_NAME, h=H, reps=1, n_taps=None, skip_off=False,
             skip_out=False, skip_in=False, staggered=False, timing=False,
             x_dt_name=None):
    dt_w = getattr(mybir.dt, dt_name)
    dt_in = getattr(mybir.dt, x_dt_name or X_DT)
    f32 = mybir.dt.float32
    bf16 = mybir.dt.bfloat16
    n_taps = N_TAPS if n_taps is None else n_taps
    nslab = h // R
    nc = bass.Bass()
    # timing=True: device-resident garbage tensors; nothing big is shipped
    # over the axon tunnel, so wall-clock differencing is low-noise.
    kio = "Internal" if timing else "ExternalInput"
    koo = "Internal" if timing else "ExternalOutput"
    x = nc.dram_tensor("x", [BPC, CIN, h, W], dt_in, kind=kio)
    off = nc.dram_tensor("off", [BPC, nslab, 128, FS], bf16, kind=kio)
    y = nc.dram_tensor("y", [BPC, nslab, 128, FS], bf16, kind=koo)
    wt = nc.dram_tensor("w", [128, len(TAPS) * COUTP], dt_w, kind=kio)
    if timing:
        tin = nc.dram_tensor("tin", [1, 64], mybir.dt.int32, kind="ExternalInput")
        tout = nc.dram_tensor("tout", [1, 64], mybir.dt.int32, kind="ExternalOutput")

    with _TileContext(nc) as tc:
        with (
            tc.tile_pool(name="wpool", bufs=1) as wpool,
            tc.tile_pool(name="slabp", bufs=2) as slabp,
            tc.tile_pool(name="offp", bufs=4) as offp,
            tc.tile_pool(name="outp", bufs=4) as outp,
            tc.tile_pool(name="psump", bufs=8, space="PSUM") as psump,
        ):
            w_t = wpool.tile([128, len(TAPS) * COUTP], dt_w, name="w_t")
            nc.sync.dma_start(w_t[:, :], wt[:, :])

            def load_slab(s):
                # slab slot j <-> input row s*R - 1 + j (R+2 slots w/ halo).
                # Loads go on SP (nc.sync); the offset load goes on ACT
                # (nc.scalar) and output stores on Pool (nc.gpsimd SWDGE) so
                # no load ever queues behind a store whose semaphore wait
                # would block the issuing sequencer.
                slab = slabp.tile([128, (R + 2) * W], dt_in, name="slab")
                r_lo = max(0, s * R - 1)
                r_hi = min(h, s * R + R + 1)
                slot0 = r_lo - (s * R - 1)
                if skip_in:  # timing-only: load a sliver so the tile allocates
                    nc.sync.dma_start(slab[:, :W], x[:, :, r_lo : r_lo + 1, :])
                else:
                    # two halves: empirically one big DMA runs slower than
                    # two queued on different logical DMA queues
                    r_mid = r_lo + (r_hi - r_lo) // 2
                    for a, b in ((r_lo, r_mid), (r_mid, r_hi)):
                        sa = slot0 + (a - r_lo)
                        nc.sync.dma_start(
                            slab[:, sa * W : (sa + (b - a)) * W],
                            x[:, :, a:b, :],
                        )
                offts = []
                for img in range(BPC):
                    off_t = offp.tile([128, FS], bf16, name="off_t")
                    hf = FS // 2
                    if not skip_off:
                        nc.sync.dma_start(off_t[:, :hf], off[img, s, :, :hf])
                        nc.sync.dma_start(off_t[:, hf:], off[img, s, :, hf:])
                    else:  # timing-only sliver
                        nc.sync.dma_start(off_t[:, :W], off[img, s, :, :W])
                    offts.append(off_t)
                return slab, offts

            if reps > 1:
                loop_kw = {}
                if staggered:
                    loop_kw = dict(
                        staggered_reset=True,
                        hint_engines=(
                            mybir.EngineType.PE,
                            mybir.EngineType.SP,
                            mybir.EngineType.DVE,
                            mybir.EngineType.Activation,
                        ),
                    )
                loop_ctx = tc.For_i(0, reps, **loop_kw)
            else:
                loop_ctx = contextlib.nullcontext()
            with loop_ctx:
                nxt = load_slab(0)
                for s in range(nslab):
                    slab, offts = nxt
                    if s + 1 < nslab:
                        nxt = load_slab(s + 1)
                    slab3 = [
                        slab[img * 64 : (img + 1) * 64, :].rearrange(
                            "p (r w) -> p r w", w=W
                        )
                        for img in range(BPC)
                    ]
                    outts = [
                        outp.tile([128, FS], bf16, name="out_t")
                        for img in range(BPC)
                    ]

                    for q in range(QPS):
                        psums = []
                        for img in range(BPC):
                            psum_t = psump.tile([128, FQ], f32, name="psum_t")
                            psums.append(psum_t)

                        # t-major emission: 8 streams (4 col-groups x 2
                        # images) advance through the taps in lockstep.
                        for ti, (kh, kw) in enumerate(TAPS[:n_taps] if n_taps else []):
                            for c in range(4):
                                for img in range(BPC):
                                    r0 = q * 8 + 2 * c
                                    gr0 = s * R + r0
                                    row_lo, nrows = 0, 2
                                    if gr0 == 0 and kh == 0:
                                        row_lo, nrows = 1, 1
                                    if gr0 == h - 2 and kh == 2:
                                        nrows = 1
                                    src_off, dst_off, ncol = KW_SPAN[kw]
                                    slot = r0 + row_lo + kh
                                    rhs = slab3[img][
                                        :, slot : slot + nrows, src_off : src_off + ncol
                                    ]
                                    out_ap = psums[img][
                                        32 * c : 32 * c + COUTP, :
                                    ].rearrange("p (r w) -> p r w", w=W)[
                                        :,
                                        row_lo : row_lo + nrows,
                                        dst_off : dst_off + ncol,
                                    ]
                                    lhsT = w_t[
                                        img * 64 : (img + 1) * 64,
                                        ti * COUTP : (ti + 1) * COUTP,
                                    ]
                                    nc.tensor.matmul(
                                        out_ap,
                                        lhsT,
                                        rhs,
                                        start=(ti == 0),
                                        stop=(ti == n_taps - 1),
                                        tile_position=(img * 64, 32 * c),
                                        # the sim's accumulation-group sanity
                                        # check mis-addresses partition-sliced
                                        # PSUM groups; its per-element
                                        # pending-zero modeling stays active.
                                        skip_group_check=True,
                                    )

                        for img in range(BPC):
                            dst = outts[img][:, q * FQ : (q + 1) * FQ]
                            osl = offts[img][:, q * FQ : (q + 1) * FQ]
                            if n_taps:
                                nc.vector.tensor_add(dst, psums[img][:, :], osl)
                            else:
                                nc.vector.tensor_copy(dst, osl)

                    if not skip_out:
                        for img in range(BPC):
                            hf = FS // 2
                            nc.sync.dma_start(y[img, s, :, :hf], outts[img][:, :hf])
                            nc.sync.dma_start(y[img, s, :, hf:], outts[img][:, hf:])
            if timing:
                tok = wpool.tile([1, 64], mybir.dt.int32, name="tok")
                nc.sync.dma_start(tok[:, :], tin[:, :])
                nc.sync.dma_start(tout[:, :], tok[:, :])
    _split_excess_waits(nc)
    return nc


def _pack_off(offb, h):
    """[n, 32, h, W] -> [n, nslab, 128, FS] in the SBUF tile layout.

    row r = s*R + q*8 + c*2 + rw maps to partition c*32+ch, free
    q*512 + rw*256 + w.
    """
    nslab = h // R
    v = offb.reshape(offb.shape[0], COUTP, nslab, QPS, 4, 2, W)
    v = v.transpose(0, 2, 4, 1, 3, 5, 6)  # n, s, c, ch, q, rw, w
    return np.ascontiguousarray(v.reshape(offb.shape[0], nslab, 128, FS))


def _unpack_y(y_dev, h):
    """[n, nslab, 128, FS] packed -> [n, COUT, h, W]."""
    n = y_dev.shape[0]
    nslab = h // R
    v = y_dev.reshape(n, nslab, 4, COUTP, QPS, 2, W)
    v = v.transpose(0, 3, 1, 4, 2, 5, 6)  # n, ch, s, q, c, rw, w
    return v.reshape(n, COUTP, h, W)[:, :COUT]


def pack_inputs(input, offset, weight, bias, dt_name=DT_NAME, h=H,
                x_dt_name=None):
    np_w = mybir.dt.np(getattr(mybir.dt, dt_name))
    np_x = mybir.dt.np(getattr(mybir.dt, x_dt_name or X_DT))
    input = np.asarray(input, dtype=np.float32)
    offset = np.asarray(offset, dtype=np.float32)
    weight = np.asarray(weight, dtype=np.float32)
    bias = np.asarray(bias, dtype=np.float32)

    np_bf16 = mybir.dt.np(mybir.dt.bfloat16)
    nimg = input.shape[0]
    offb = np.zeros((nimg, COUTP, h, W), dtype=np.float32)
    offb[:, :COUT] = offset[:, :COUT, :h] + bias[None, :, None, None]
    off_packed = _pack_off(offb, h).astype(np_bf16)
    w_packed = np.zeros((128, len(TAPS) * COUTP), dtype=np_w)
    for t, (kh, kw) in enumerate(TAPS):
        w_packed[0:64, t * COUTP : t * COUTP + COUT] = (
            weight[:, :, kh, kw].T / XS
        ).astype(np_w)
    w_packed[64:128] = w_packed[0:64]
    xc = np.clip(input * XS, -15.5, 15.5).astype(np_x)
    in_maps = [
        {
            "x": np.ascontiguousarray(xc[BPC * k : BPC * (k + 1), :, :h]),
            "off": off_packed[BPC * k : BPC * (k + 1)],
            "w": w_packed,
        }
        for k in range(nimg // BPC)
    ]
    return in_maps


_NC_CACHE = {}


def run_on_hw(input, offset, weight, bias, dt_name=DT_NAME, trace=False):
    key = dt_name
    if key not in _NC_CACHE:
        _NC_CACHE[key] = build_nc(dt_name)
    nc = _NC_CACHE[key]
    in_maps = pack_inputs(input, offset, weight, bias, dt_name)
    res = run_bass_kernel_spmd(nc, in_maps, list(range(NCORES)), trace=trace)
    y_dev = np.concatenate([res.results[k]["y"] for k in range(NCORES)], axis=0)
    out = _unpack_y(y_dev, H)
    return np.ascontiguousarray(out.astype(np.float32, copy=False)), res


def kernel(input, offset, weight, bias):
    out, _ = run_on_hw(input, offset, weight, bias)
    return out



# revision 67
# speedup vs baseline: 1.0270x; 1.0270x over previous
"""Trainium2 Bass kernel for nn_DeformConv2d_72765335929324.

The module is a dense 3x3 conv (stride 1, pad 1) [B,64,256,256] -> [B,18,256,256]
plus a per-pixel additive `offset` term and a channel bias.

Strategy (per core; batch is sharded 2 images/core across 8 cores):
- conv = 9 taps, each a [cin=64 -> cout=18] matmul over shifted input views.
- PE array packing via tile_position: 2 images on row-groups {0,64} x 4
  output row-pair chunks on col-groups {0,32,64,96} -> 8 concurrent matmul
  streams, 9 accumulating taps each, N=512 (2 output rows) per stream.
- The moving operand (input) is float8e3: the PE runs fp8 at bf16 speed, so
  this halves input HBM traffic for free; weights stay bf16 (mixed-dtype
  matmul is supported and HW-exact vs the quantized reference). x is
  pre-scaled by 2 (w by 1/2) to keep small values out of e3m4 subnormals
  while staying under the e3m4 max normal of 15.5 (clipped for safety).
- offset+bias are pre-added on the host, packed into the SBUF tile layout
  and shipped as bf16; output is returned as bf16 (~2x less off/out HBM
  traffic vs f32, rel-err ~4.6e-3 vs the 2e-2 budget).
- All bulk transfers are full-width [128, *] DMAs split into ~0.5-2MB
  halves so they spread across DMA queues; loads issue on the SP HWDGE
  ring, stores on the ACT ring so a store's adds-done wait can't
  head-of-line-block the next slab's loads (A/B-tested best; coarser
  single-DMA-per-slab variants and deeper prefetch measured slower or
  neutral).
- W-edge zero-padding is realized by shrinking the matmul N-span per kw tap;
  H-edge padding by shrinking the row span of the first/last chunk taps.
"""

import contextlib
import os
import numpy as np

import concourse.bass as bass
import concourse.tile as tile
import concourse.mybir as mybir
from concourse.vector_clock import ScopedClock
from concourse.bass_utils import run_bass_kernel_spmd

B, CIN, H, W = 16, 64, 256, 256
COUT = 18
COUTP = 32  # cout padded to a 32-wide PE column group
NCORES = 8
BPC = B // NCORES  # images per core
R = 64  # output rows per input slab
NSLAB = H // R
QPS = R // 8  # quad chunks per slab (each quad = 8 rows)
FQ = 2 * W  # free size of one quad chunk (2 rows x 256 cols)
FS = QPS * FQ  # free size of one slab tile

# tap order: full-coverage center tap first so start=True initializes the
# whole PSUM bank region before partial-coverage taps accumulate.
TAPS = [(1, 1), (0, 0), (0, 1), (0, 2), (1, 0), (1, 2), (2, 0), (2, 1), (2, 2)]

# kw -> (src col offset, dst col offset, ncols): zero-pad at W edges is
# realized by shrinking the span instead of padding SBUF.
KW_SPAN = {0: (0, 1, W - 1), 1: (0, 0, W), 2: (1, 0, W - 1)}

DT_NAME = os.environ.get("CONV_DT", "bfloat16")
# input (moving operand) dtype: float8e3 halves input HBM traffic; the
# PE runs fp8 at bf16 speed so only DMA changes. x is pre-scaled by XS
# (and w by 1/XS, exact in bf16) to push small values out of the e3m4
# subnormal range; e3m4 max normal is 31 so |4x| <= ~24 is safe.
X_DT = os.environ.get("CONV_XDT", "float8e3")
XS = 2.0  # e3m4 max normal is 15.5: |2x| <= ~11 for this data, clipped anyway
# timing experiments only: restrict the tap count (wrong results!)
N_TAPS = int(os.environ.get("CONV_TAPS", "9"))


class _TileContext(tile.TileContext):
    """TileContext whose tail drain spreads its semaphore waits over NOPs.

    The stock _drain_and_barrier puts one wait per logical proc on a single
    Drain instruction; the walrus build here rejects instructions carrying
    more than 1-2 sync waits.
    """

    def _drain_and_barrier(self, tick_clock, wait_clock):
        nc = self.nc
        carriers = [nc.sync.nop(nofuse=True) for _ in range(64)]
        drain_inst = nc.sync.drain()
        wait_clock.add_sem_waits(
            drain_inst.ins, ScopedClock({None: tick_clock.global_clock})
        )
        si = drain_inst.ins.sync_info
        waits = list(si.on_wait or []) if si is not None else []
        if len(waits) > 1:
            si.on_wait = waits[:1]
            extra = waits[1:]
            assert len(extra) <= len(carriers)
            for wt, nop in zip(extra, carriers):
                nsi = nop.ins.sync_info
                if nsi is None:
                    nop.ins.sync_info = mybir.SyncInfo(on_wait=[wt], on_update=[])
                else:
                    nsi.on_wait = [wt]
        nc.all_engine_barrier()
        assert self.sems is not None
        popped = nc._tile_sem_poison_stack.pop()
        assert popped is self._sem_poison
        nc.clear_and_free_semaphores(list(self.sems.allocated().values()))
        nc.all_engine_barrier()


def _split_excess_waits(nc):
    """Spill per-instruction semaphore waits onto same-engine NOP carriers.

    Tile's wait assigner attaches up to ~6 waits to one instruction; the
    walrus build here rejects >1 sync wait on engine instructions (>2 on
    EventSemaphore). A NOP that runs just before the instruction on the same
    engine is semantically equivalent (program order on one engine is
    serial). For DMAs, the wait kept in-descriptor is evaluated by the DGE
    without stalling the issuing engine, so keep the freshest (engine-sem)
    wait there and spill the long-satisfied WAR waits on old DMA completions.
    """
    for bb in nc.m.functions[0].blocks:
        new = []
        for inst in bb.instructions:
            si = inst.sync_info
            waits = list(si.on_wait) if si and si.on_wait else []
            cap = 2 if isinstance(inst, mybir.InstEventSemaphore) else 1
            if len(waits) > cap:
                if isinstance(inst, mybir.InstDMACopy):
                    waits.sort(key=lambda w: ((w.ant_name or "").startswith("DMA"),))
                si.on_wait = waits[:cap]
                for w in waits[cap:]:
                    n = mybir.InstNoOp(
                        name=nc.get_next_instruction_name(), ins=[], outs=[]
                    )
                    n.engine = inst.engine
                    n.sync_info = mybir.SyncInfo(on_wait=[w], on_update=[])
                    new.append(n)
            new.append(inst)
        bb.instructions = new


def build_nc(dt_name=DT_NAME, h=H, reps=1, n_taps=None, skip_off=False,
             skip_out=False, skip_in=False, staggered=False, timing=False,
             x_dt_name=None, store_eng="scalar", off_eng="sync", early_store=0,
             offb=4, outb=4, slabb=2, ahead=1):
    dt_w = getattr(mybir.dt, dt_name)
    dt_in = getattr(mybir.dt, x_dt_name or X_DT)
    f32 = mybir.dt.float32
    bf16 = mybir.dt.bfloat16
    n_taps = N_TAPS if n_taps is None else n_taps
    nslab = h // R
    nc = bass.Bass()
    # timing=True: device-resident garbage tensors; nothing big is shipped
    # over the axon tunnel, so wall-clock differencing is low-noise.
    kio = "Internal" if timing else "ExternalInput"
    koo = "Internal" if timing else "ExternalOutput"
    x = nc.dram_tensor("x", [BPC, CIN, h, W], dt_in, kind=kio)
    off = nc.dram_tensor("off", [BPC, nslab, 128, FS], bf16, kind=kio)
    y = nc.dram_tensor("y", [BPC, nslab, 128, FS], bf16, kind=koo)
    wt = nc.dram_tensor("w", [128, len(TAPS) * COUTP], dt_w, kind=kio)
    if timing:
        tin = nc.dram_tensor("tin", [1, 64], mybir.dt.int32, kind="ExternalInput")
        tout = nc.dram_tensor("tout", [1, 64], mybir.dt.int32, kind="ExternalOutput")

    st = getattr(nc, store_eng)
    oe = getattr(nc, off_eng)
    with _TileContext(nc) as tc:
        with (
            tc.tile_pool(name="wpool", bufs=1) as wpool,
            tc.tile_pool(name="slabp", bufs=slabb) as slabp,
            tc.tile_pool(name="offp", bufs=offb) as offp,
            tc.tile_pool(name="outp", bufs=outb) as outp,
            tc.tile_pool(name="psump", bufs=8, space="PSUM") as psump,
        ):
            w_t = wpool.tile([128, len(TAPS) * COUTP], dt_w, name="w_t")
            nc.sync.dma_start(w_t[:, :], wt[:, :])

            def load_slab(s):
                # slab slot j <-> input row s*R - 1 + j (R+2 slots w/ halo).
                # Loads go on SP (nc.sync); the offset load goes on ACT
                # (nc.scalar) and output stores on Pool (nc.gpsimd SWDGE) so
                # no load ever queues behind a store whose semaphore wait
                # would block the issuing sequencer.
                slab = slabp.tile([128, (R + 2) * W], dt_in, name="slab")
                r_lo = max(0, s * R - 1)
                r_hi = min(h, s * R + R + 1)
                slot0 = r_lo - (s * R - 1)
                if skip_in:  # timing-only: load a sliver so the tile allocates
                    nc.sync.dma_start(slab[:, :W], x[:, :, r_lo : r_lo + 1, :])
                else:
                    # two halves: empirically one big DMA runs slower than
                    # two queued on different logical DMA queues
                    r_mid = r_lo + (r_hi - r_lo) // 2
                    for a, b in ((r_lo, r_mid), (r_mid, r_hi)):
                        sa = slot0 + (a - r_lo)
                        nc.sync.dma_start(
                            slab[:, sa * W : (sa + (b - a)) * W],
                            x[:, :, a:b, :],
                        )
                offts = []
                for img in range(BPC):
                    off_t = offp.tile([128, FS], bf16, name="off_t")
                    hf = FS // 2
                    if not skip_off:
                        oe.dma_start(off_t[:, :hf], off[img, s, :, :hf])
                        oe.dma_start(off_t[:, hf:], off[img, s, :, hf:])
                    else:  # timing-only sliver
                        oe.dma_start(off_t[:, :W], off[img, s, :, :W])
                    offts.append(off_t)
                return slab, offts

            if reps > 1:
                loop_kw = {}
                if staggered:
                    loop_kw = dict(
                        staggered_reset=True,
                        hint_engines=(
                            mybir.EngineType.PE,
                            mybir.EngineType.SP,
                            mybir.EngineType.DVE,
                            mybir.EngineType.Activation,
                        ),
                    )
                loop_ctx = tc.For_i(0, reps, **loop_kw)
            else:
                loop_ctx = contextlib.nullcontext()
            with loop_ctx:
                # prefetch `ahead` slabs: with ahead=2 the loads for s+2 sit
                # in the DMA ring BEFORE the stores of slab s, so a store's
                # adds-done wait can never head-of-line-block them.
                pend = [load_slab(i) for i in range(min(ahead, nslab))]
                for s in range(nslab):
                    slab, offts = pend.pop(0)
                    if s + ahead < nslab:
                        pend.append(load_slab(s + ahead))
                    slab3 = [
                        slab[img * 64 : (img + 1) * 64, :].rearrange(
                            "p (r w) -> p r w", w=W
                        )
                        for img in range(BPC)
                    ]
                    outts = [
                        outp.tile([128, FS], bf16, name="out_t")
                        for img in range(BPC)
                    ]

                    for q in range(QPS):
                        psums = []
                        for img in range(BPC):
                            psum_t = psump.tile([128, FQ], f32, name="psum_t")
                            psums.append(psum_t)

                        # t-major emission: 8 streams (4 col-groups x 2
                        # images) advance through the taps in lockstep.
                        for ti, (kh, kw) in enumerate(TAPS[:n_taps] if n_taps else []):
                            for c in range(4):
                                for img in range(BPC):
                                    r0 = q * 8 + 2 * c
                                    gr0 = s * R + r0
                                    row_lo, nrows = 0, 2
                                    if gr0 == 0 and kh == 0:
                                        row_lo, nrows = 1, 1
                                    if gr0 == h - 2 and kh == 2:
                                        nrows = 1
                                    src_off, dst_off, ncol = KW_SPAN[kw]
                                    slot = r0 + row_lo + kh
                                    rhs = slab3[img][
                                        :, slot : slot + nrows, src_off : src_off + ncol
                                    ]
                                    out_ap = psums[img][
                                        32 * c : 32 * c + COUTP, :
                                    ].rearrange("p (r w) -> p r w", w=W)[
                                        :,
                                        row_lo : row_lo + nrows,
                                        dst_off : dst_off + ncol,
                                    ]
                                    lhsT = w_t[
                                        img * 64 : (img + 1) * 64,
                                        ti * COUTP : (ti + 1) * COUTP,
                                    ]
                                    nc.tensor.matmul(
                                        out_ap,
                                        lhsT,
                                        rhs,
                                        start=(ti == 0),
                                        stop=(ti == n_taps - 1),
                                        tile_position=(img * 64, 32 * c),
                                        # the sim's accumulation-group sanity
                                        # check mis-addresses partition-sliced
                                        # PSUM groups; its per-element
                                        # pending-zero modeling stays active.
                                        skip_group_check=True,
                                    )

                        for img in range(BPC):
                            dst = outts[img][:, q * FQ : (q + 1) * FQ]
                            osl = offts[img][:, q * FQ : (q + 1) * FQ]
                            if n_taps:
                                nc.vector.tensor_add(dst, psums[img][:, :], osl)
                            else:
                                nc.vector.tensor_copy(dst, osl)

                        # early_store: ship each out half as soon as the
                        # quads covering it are added, so store wire time
                        # overlaps the rest of the slab's compute.
                        if early_store and not skip_out and q == QPS // 2 - 1:
                            for img in range(BPC):
                                hf = FS // 2
                                st.dma_start(
                                    y[img, s, :, :hf], outts[img][:, :hf]
                                )

                    if not skip_out:
                        for img in range(BPC):
                            hf = FS // 2
                            if not early_store:
                                st.dma_start(y[img, s, :, :hf], outts[img][:, :hf])
                            st.dma_start(y[img, s, :, hf:], outts[img][:, hf:])
            if timing:
                tok = wpool.tile([1, 64], mybir.dt.int32, name="tok")
                nc.sync.dma_start(tok[:, :], tin[:, :])
                nc.sync.dma_start(tout[:, :], tok[:, :])
    _split_excess_waits(nc)
    return nc


def _pack_off(offb, h):
    """[n, 32, h, W] -> [n, nslab, 128, FS] in the SBUF tile layout.

    row r = s*R + q*8 + c*2 + rw maps to partition c*32+ch, free
    q*512 + rw*256 + w.
    """
    nslab = h // R
    v = offb.reshape(offb.shape[0], COUTP, nslab, QPS, 4, 2, W)
    v = v.transpose(0, 2, 4, 1, 3, 5, 6)  # n, s, c, ch, q, rw, w
    return np.ascontiguousarray(v.reshape(offb.shape[0], nslab, 128, FS))


def _unpack_y(y_dev, h):
    """[n, nslab, 128, FS] packed -> [n, COUT, h, W]."""
    n = y_dev.shape[0]
    nslab = h // R
    v = y_dev.reshape(n, nslab, 4, COUTP, QPS, 2, W)
    v = v.transpose(0, 3, 1, 4, 2, 5, 6)  # n, ch, s, q, c, rw, w
    return v.reshape(n, COUTP, h, W)[:, :COUT]


def pack_inputs(input, offset, weight, bias, dt_name=DT_NAME, h=H,
                x_dt_name=None):
    np_w = mybir.dt.np(getattr(mybir.dt, dt_name))
    np_x = mybir.dt.np(getattr(mybir.dt, x_dt_name or X_DT))
    input = np.asarray(input, dtype=np.float32)
    offset = np.asarray(offset, dtype=np.float32)
    weight = np.asarray(weight, dtype=np.float32)
    bias = np.asarray(bias, dtype=np.float32)

    np_bf16 = mybir.dt.np(mybir.dt.bfloat16)
    nimg = input.shape[0]
    offb = np.zeros((nimg, COUTP, h, W), dtype=np.float32)
    offb[:, :COUT] = offset[:, :COUT, :h] + bias[None, :, None, None]
    off_packed = _pack_off(offb, h).astype(np_bf16)
    w_packed = np.zeros((128, len(TAPS) * COUTP), dtype=np_w)
    for t, (kh, kw) in enumerate(TAPS):
        w_packed[0:64, t * COUTP : t * COUTP + COUT] = (
            weight[:, :, kh, kw].T / XS
        ).astype(np_w)
    w_packed[64:128] = w_packed[0:64]
    xc = np.clip(input * XS, -15.5, 15.5).astype(np_x)
    in_maps = [
        {
            "x": np.ascontiguousarray(xc[BPC * k : BPC * (k + 1), :, :h]),
            "off": off_packed[BPC * k : BPC * (k + 1)],
            "w": w_packed,
        }
        for k in range(nimg // BPC)
    ]
    return in_maps


_NC_CACHE = {}


def run_on_hw(input, offset, weight, bias, dt_name=DT_NAME, trace=False):
    key = dt_name
    if key not in _NC_CACHE:
        _NC_CACHE[key] = build_nc(dt_name)
    nc = _NC_CACHE[key]
    in_maps = pack_inputs(input, offset, weight, bias, dt_name)
    res = run_bass_kernel_spmd(nc, in_maps, list(range(NCORES)), trace=trace)
    y_dev = np.concatenate([res.results[k]["y"] for k in range(NCORES)], axis=0)
    out = _unpack_y(y_dev, H)
    return np.ascontiguousarray(out.astype(np.float32, copy=False)), res


def kernel(input, offset, weight, bias):
    out, _ = run_on_hw(input, offset, weight, bias)
    return out

